# revision 12
# baseline (speedup 1.0000x reference)
"""BoxRenderLoss Trainium2 kernel (v4: rows-on-partitions, 10-pt coord grids).

loss = mean over (box, fragment) pairs of masked min-squared-distance between
each box's 10x10 fragment grid and the other box's 100-point sampled boundary,
both directions, / (2*B*FP).

Math: the min over the 100 boundary points decomposes into the 4 box edges;
each edge's 25-point uniform grid min has the closed form
k = clamp(round(u/s), 0, 24), val = u - s*k = -s*(k - T), T = u/s.  Key
structural win: the fragment grid is a 10x10 product grid, so every
per-coordinate quantity (u, v, u^2, v^2, val^2, min(u,v)) takes only 10
distinct values per axis per row; only the final combine runs over the full
100 fragments, via stride-0 broadcast access patterns.

The outside-mask multiply is folded into the min-chain:
  contribution = relu(min(e1, e2, -BIG*min(ux,vx,uy,vy)))
with BIG = 2^13: inside fragments give a nonpositive third term -> relu -> 0;
outside ones a huge positive one -> min picks dmin.  The final relu+row-sum is
a 4x-mode DVE tensor_scalar with accum_out; a K=128 PE matmul against a ones
vector then collapses the 128 partials so the output DMA is one descriptor
(a [128,1] output costs ~9us of per-descriptor semaphore propagation).

All matmul weights are bf16 in ONE DMA; T = u*(24/tw) needs ~f32 accuracy
(wrx, dxrx cancel), so its lhsT carries hi/lo bf16 pairs (K=8 split matmul),
which beats f32r's 4-pass LdWeights+Matmult (213+213ns per block) ~5x.

Device layout: partitions = 128 virtual rows (4096 boxes x 2 directions,
data-parallel over 8 cores -> 1024 rows/core = 8 blocks of 128), free dim =
8 blocks x (10 x-grid | 10 y-grid) = 160 for coordinate ops, 8 x 10 x 10 =
800 for fragment-product ops.
"""

import os
import numpy as np
import ml_dtypes

# Exact float32 bit patterns of jnp.linspace(0.0, 1.0, 10) (fragment grid).
_LIN10 = np.array(
    [0, 1038323257, 1046711865, 1051372203, 1055100473,
     1057896676, 1059760811, 1061624946, 1063489081, 1065353216],
    dtype=np.uint32,
).view(np.float32)

_B = 4096
_FP = 100
_N_CORES = 8
_BOX_PER_CORE = _B // _N_CORES          # 512
_ROWS = 2 * _BOX_PER_CORE               # 1024 virtual rows per core
_NBLK = _ROWS // 128                    # 8 blocks of 128 partitions
_CW = 20                                # coord cols per block (10 x | 10 y)
_CF = _NBLK * _CW                       # 160 coord cols total
_PF = _NBLK * 100                       # 800 product cols total
_MAGIC = 8388608.0                      # 2^23 round-to-nearest trick
_BIG = 8192.0                           # 2^13 mask scale

LAST_RESULTS = None  # BassKernelResults of the most recent run (for test.py)

_compiled = {}


def _build_nc():
    import concourse.bass as bass
    import concourse.bacc as bacc
    import concourse.tile as tile
    from concourse import mybir

    f32 = mybir.dt.float32
    bf16 = mybir.dt.bfloat16
    Op = mybir.AluOpType
    Act = mybir.ActivationFunctionType

    nc = bacc.Bacc("TRN2", target_bir_lowering=False, debug=False,
                   num_devices=_N_CORES)
    # wts: per block b at col b*512: U lhsT (rows 0-3: w,dx,h,dy) | V lhsT
    # (rows 0-3) | Un lhsT (rows 0-3) | T lhsT (rows 0-7: hi/lo pairs).
    # wts: trailing 40 cols hold the rhs (cols 0-19 for U/V/Un rows 0-3,
    # cols 20-39 for T rows 0-7).
    wts_d = nc.dram_tensor("wts", [8, _NBLK * 512 + 40], bf16,
                           kind="ExternalInput").ap()
    s2t_d = nc.dram_tensor("s2t", [128, _CF], bf16, kind="ExternalInput").ap()
    out_d = nc.dram_tensor("out", [2, 1], f32, kind="ExternalOutput").ap()

    with tile.TileContext(nc) as tc:
        with (
            tc.tile_pool(name="const", bufs=1) as const,
            tc.tile_pool(name="sb", bufs=1) as sb,
            tc.tile_pool(name="ps", bufs=1, space="PSUM") as ps,
        ):
            wts = const.tile([8, _NBLK * 512 + 40], bf16)
            s2t = const.tile([128, _CF], bf16)
            # T lhsT slices + rhs first: the T->kc0 chain is critical
            for b in range(_NBLK):
                l0 = b * 512
                nc.sync.dma_start(wts[0:8, l0 + 384:l0 + 512],
                                  wts_d[0:8, l0 + 384:l0 + 512])
            nc.sync.dma_start(wts[0:8, _NBLK * 512:], wts_d[0:8, _NBLK * 512:])
            for b in range(_NBLK):
                l0 = b * 512
                nc.sync.dma_start(wts[0:4, l0:l0 + 384],
                                  wts_d[0:4, l0:l0 + 384])
            nc.sync.dma_start(s2t[:], s2t_d[:])
            cb = const.tile([128, 1], f32)
            nc.gpsimd.memset(cb[:], -_MAGIC)
            ones = const.tile([128, 1], f32)
            nc.gpsimd.memset(ones[:], 1.0)
            part = const.tile([128, 2], f32)

            U = ps.tile([128, _CF], f32, tag="U")
            V = ps.tile([128, _CF], f32, tag="V")
            Un = ps.tile([128, _CF], f32, tag="Un")
            T = ps.tile([128, _CF], f32, tag="T")
            for b in range(_NBLK):
                cs = slice(b * _CW, (b + 1) * _CW)
                l0 = b * 512
                nc.tensor.matmul(T[:, cs], wts[0:8, l0 + 384:l0 + 512],
                                 wts[0:8, _NBLK * 512 + 20:_NBLK * 512 + 40])
            for b in range(_NBLK):
                cs = slice(b * _CW, (b + 1) * _CW)
                l0 = b * 512
                rr = slice(_NBLK * 512, _NBLK * 512 + 20)
                nc.tensor.matmul(U[:, cs], wts[0:4, l0:l0 + 128], wts[0:4, rr])
                nc.tensor.matmul(V[:, cs], wts[0:4, l0 + 128:l0 + 256],
                                 wts[0:4, rr])
                nc.tensor.matmul(Un[:, cs], wts[0:4, l0 + 256:l0 + 384],
                                 wts[0:4, rr])

            # --- coordinate-level ops, [128, 160] ---
            # k = clamp(round(T), 0, 24); d = k - T; val^2 = s^2 * d^2
            kc0 = sb.tile([128, _CF], f32, tag="kc0")
            nc.vector.tensor_scalar(kc0[:], T[:], _MAGIC, _MAGIC + 24.0,
                                    Op.add, Op.min)
            kc1 = sb.tile([128, _CF], bf16, tag="kc1")
            nc.scalar.activation(kc1[:], kc0[:], Act.Relu, bias=cb[:, 0:1])
            d = sb.tile([128, _CF], bf16, tag="d")
            nc.vector.tensor_tensor(d[:], kc1[:], T[:], Op.subtract)
            dsq = sb.tile([128, _CF], bf16, tag="dsq")
            nc.scalar.activation(dsq[:], d[:], Act.Square)
            vq = sb.tile([128, _CF], bf16, tag="vq")
            nc.vector.tensor_tensor(vq[:], dsq[:], s2t[:], Op.mult)

            usq = sb.tile([128, _CF], bf16, tag="usq")
            nc.scalar.activation(usq[:], U[:], Act.Square)
            vsq = sb.tile([128, _CF], bf16, tag="vsq")
            nc.scalar.activation(vsq[:], V[:], Act.Square)
            exy = sb.tile([128, _CF], bf16, tag="exy")
            nc.vector.tensor_tensor(exy[:], usq[:], vsq[:], Op.min)

            # mask path: m1n = max(-BIG*u, -BIG*v) = -BIG*min(u,v)
            vsn = sb.tile([128, _CF], bf16, tag="vsn")
            nc.scalar.activation(vsn[:], V[:], Act.Copy, scale=-_BIG)
            m1n = sb.tile([128, _CF], bf16, tag="m1n")
            nc.vector.tensor_tensor(m1n[:], Un[:], vsn[:], Op.max)

            # --- fragment-product ops, [128, 800] via broadcast views ---
            def cview(t, c, inner_j):
                # [128, 8, 10, 10] view of coord half c (0=x grid i, 1=y grid j)
                a = t[:].rearrange("p (b c t) -> p b c t", b=_NBLK, c=2)
                a = a[:, :, c, :]              # [128, 8, 10]
                if inner_j:                    # values indexed by j (inner)
                    a = a.unsqueeze(2)         # [128, 8, 1, 10]
                else:                          # values indexed by i (outer)
                    a = a.unsqueeze(3)         # [128, 8, 10, 1]
                return a.broadcast_to((128, _NBLK, 10, 10))

            def pview(t):
                return t[:].rearrange("p (b i j) -> p b i j", i=10, j=10)

            # mmx = relu(-BIG * min over the 4 coord margins); the relu
            # rides the Act-materialized packed x-repeat (relu(max(a,b)) ==
            # max(b, relu(a))), so the max runs in 2x DVE mode.  Since
            # dmin >= 0, contribution = relu(min(e1,e2,raw)) = min(dmin,mmx).
            e1 = sb.tile([128, _PF], bf16, tag="e1")
            nc.vector.tensor_tensor(pview(e1), cview(vq, 1, True),
                                    cview(exy, 0, False), Op.add)
            e2 = sb.tile([128, _PF], bf16, tag="e2")
            nc.vector.tensor_tensor(pview(e2), cview(vq, 0, False),
                                    cview(exy, 1, True), Op.add)
            m1xr = sb.tile([128, _PF], bf16, tag="m1xr")
            nc.scalar.activation(pview(m1xr), cview(m1n, 0, False), Act.Relu)
            mmx = sb.tile([128, _PF], bf16, tag="mmx")
            nc.vector.tensor_tensor(pview(mmx), cview(m1n, 1, True),
                                    pview(m1xr), Op.max)
            dmin = sb.tile([128, _PF], bf16, tag="dmin")
            nc.vector.tensor_tensor(dmin[:], e1[:], e2[:], Op.min)
            tm = sb.tile([128, _PF], bf16, tag="tm")
            nc.vector.tensor_tensor(tm[:], dmin[:], mmx[:], Op.min)
            # final sums: halves on different engines so they run in parallel
            scr = sb.tile([128, _PF], bf16, tag="scr")
            nc.scalar.activation(scr[:, 0:_PF // 2], tm[:, 0:_PF // 2],
                                 Act.Identity, accum_out=part[:, 0:1])
            nc.vector.tensor_scalar(scr[:, _PF // 2:], tm[:, _PF // 2:], 0.0,
                                    None, Op.add, Op.add,
                                    accum_out=part[:, 1:2])

            # collapse 128 per-partition partials -> [1,1] so the output DMA
            # is a single descriptor
            sm = ps.tile([2, 1], f32, tag="sm")
            nc.tensor.matmul(sm[0:2, 0:1], part[:, 0:2], ones[:, 0:1])
            smc = sb.tile([2, 1], f32, tag="smc")
            nc.scalar.activation(smc[0:2, 0:1], sm[0:2, 0:1], Act.Copy)
            nc.sync.dma_start(out_d[:], smc[0:2, 0:1])
    nc.compile()
    return nc


def _bf16_hilo(v):
    bf = ml_dtypes.bfloat16
    hi = v.astype(bf)
    lo = (v - hi.astype(np.float32)).astype(bf)
    return hi, lo


def _core_inputs(boxes_c, targets_c):
    """Build the per-core DRAM input map (512 boxes -> 1024 virtual rows)."""
    A = np.concatenate([boxes_c, targets_c]).astype(np.float32)   # frag source
    T = np.concatenate([targets_c, boxes_c]).astype(np.float32)   # grid box
    w = A[:, 2] - A[:, 0]
    h = A[:, 3] - A[:, 1]
    tw = T[:, 2] - T[:, 0]
    th = T[:, 3] - T[:, 1]
    dx = A[:, 0] - T[:, 0]
    dy = A[:, 1] - T[:, 1]
    dvx = T[:, 2] - A[:, 0]
    dvy = T[:, 3] - A[:, 1]
    with np.errstate(divide="ignore"):
        rix = np.where(tw != 0, np.float32(24.0) / tw, np.float32(0.0))
        riy = np.where(th != 0, np.float32(24.0) / th, np.float32(0.0))
    rix = rix.astype(np.float32)
    riy = riy.astype(np.float32)
    nbig = np.float32(-_BIG)

    bf = ml_dtypes.bfloat16
    wts = np.zeros((8, _NBLK * 512), dtype=np.float32)
    for b in range(_NBLK):
        rs = slice(b * 128, (b + 1) * 128)
        l0 = b * 512
        wts[0, l0:l0 + 128] = w[rs]
        wts[1, l0:l0 + 128] = dx[rs]
        wts[2, l0:l0 + 128] = h[rs]
        wts[3, l0:l0 + 128] = dy[rs]
        wts[0, l0 + 128:l0 + 256] = -w[rs]
        wts[1, l0 + 128:l0 + 256] = dvx[rs]
        wts[2, l0 + 128:l0 + 256] = -h[rs]
        wts[3, l0 + 128:l0 + 256] = dvy[rs]
        wts[0, l0 + 256:l0 + 384] = nbig * w[rs]
        wts[1, l0 + 256:l0 + 384] = nbig * dx[rs]
        wts[2, l0 + 256:l0 + 384] = nbig * h[rs]
        wts[3, l0 + 256:l0 + 384] = nbig * dy[rs]
    wtsb = wts.astype(bf)
    for b in range(_NBLK):
        rs = slice(b * 128, (b + 1) * 128)
        l0 = b * 512
        for row, v in ((0, w * rix), (2, dx * rix),
                       (4, h * riy), (6, dy * riy)):
            hi, lo = _bf16_hilo(v[rs].astype(np.float32))
            wtsb[row, l0 + 384:l0 + 512] = hi
            wtsb[row + 1, l0 + 384:l0 + 512] = lo

    rhs = np.zeros((8, 40), dtype=np.float32)
    rhs[0, 0:10] = _LIN10
    rhs[1, 0:10] = 1.0
    rhs[2, 10:20] = _LIN10
    rhs[3, 10:20] = 1.0
    rhs[0, 20:30] = _LIN10
    rhs[1, 20:30] = _LIN10
    rhs[2, 20:30] = 1.0
    rhs[3, 20:30] = 1.0
    rhs[4, 30:40] = _LIN10
    rhs[5, 30:40] = _LIN10
    rhs[6, 30:40] = 1.0
    rhs[7, 30:40] = 1.0

    sx = tw / np.float32(24.0)
    sy = th / np.float32(24.0)
    s2 = np.zeros((128, _NBLK, 2, 10), dtype=np.float32)
    for b in range(_NBLK):
        rs = slice(b * 128, (b + 1) * 128)
        s2[:, b, 0, :] = (sx[rs] * sx[rs])[:, None]
        s2[:, b, 1, :] = (sy[rs] * sy[rs])[:, None]

    return {
        "wts": np.concatenate([wtsb, rhs.astype(bf)], axis=1),
        "s2t": s2.reshape(128, _CF).astype(bf),
    }


def kernel(boxes: np.ndarray, targets: np.ndarray) -> np.ndarray:
    from concourse.bass_utils import run_bass_kernel_spmd

    global LAST_RESULTS
    boxes = np.ascontiguousarray(boxes, dtype=np.float32)
    targets = np.ascontiguousarray(targets, dtype=np.float32)
    assert boxes.shape == (_B, 4) and targets.shape == (_B, 4)

    if "nc" not in _compiled:
        _compiled["nc"] = _build_nc()
    nc = _compiled["nc"]

    in_maps = []
    for c in range(_N_CORES):
        rows = slice(c * _BOX_PER_CORE, (c + 1) * _BOX_PER_CORE)
        in_maps.append(_core_inputs(boxes[rows], targets[rows]))

    trace = bool(int(os.environ.get("BOXLOSS_TRACE", "0")))
    res = run_bass_kernel_spmd(nc, in_maps, list(range(_N_CORES)),
                               trace=trace)
    LAST_RESULTS = res

    total = np.float64(0.0)
    for r in res.results:
        total += r["out"].astype(np.float64).sum()
    loss = total / (2.0 * _B * _FP)
    return np.array(loss, dtype=np.float32)


# revision 13
# speedup vs baseline: 1.3610x; 1.3610x over previous
"""BoxRenderLoss Trainium2 kernel (v4: rows-on-partitions, 10-pt coord grids).

loss = mean over (box, fragment) pairs of masked min-squared-distance between
each box's 10x10 fragment grid and the other box's 100-point sampled boundary,
both directions, / (2*B*FP).

Math: the min over the 100 boundary points decomposes into the 4 box edges;
each edge's 25-point uniform grid min has the closed form
k = clamp(round(u/s), 0, 24), val = u - s*k = -s*(k - T), T = u/s.  Key
structural win: the fragment grid is a 10x10 product grid, so every
per-coordinate quantity (u, v, u^2, v^2, val^2, min(u,v)) takes only 10
distinct values per axis per row; only the final combine runs over the full
100 fragments, via stride-0 broadcast access patterns.

The outside-mask multiply is folded into the min-chain:
  contribution = relu(min(e1, e2, -BIG*min(ux,vx,uy,vy)))
with BIG = 2^13: inside fragments give a nonpositive third term -> relu -> 0;
outside ones a huge positive one -> min picks dmin.  The final relu+row-sum is
a 4x-mode DVE tensor_scalar with accum_out; a K=128 PE matmul against a ones
vector then collapses the 128 partials so the output DMA is one descriptor
(a [128,1] output costs ~9us of per-descriptor semaphore propagation).

All matmul weights are bf16 in ONE DMA; T = u*(24/tw) needs ~f32 accuracy
(wrx, dxrx cancel), so its lhsT carries hi/lo bf16 pairs (K=8 split matmul),
which beats f32r's 4-pass LdWeights+Matmult (213+213ns per block) ~5x.

Device layout: partitions = 128 virtual rows (4096 boxes x 2 directions,
data-parallel over 8 cores -> 1024 rows/core = 8 blocks of 128), free dim =
8 blocks x (10 x-grid | 10 y-grid) = 160 for coordinate ops, 8 x 10 x 10 =
800 for fragment-product ops.
"""

import os
import numpy as np
import ml_dtypes

# Exact float32 bit patterns of jnp.linspace(0.0, 1.0, 10) (fragment grid).
_LIN10 = np.array(
    [0, 1038323257, 1046711865, 1051372203, 1055100473,
     1057896676, 1059760811, 1061624946, 1063489081, 1065353216],
    dtype=np.uint32,
).view(np.float32)

_B = 4096
_FP = 100
_N_CORES = 8
_BOX_PER_CORE = _B // _N_CORES          # 512
_ROWS = 2 * _BOX_PER_CORE               # 1024 virtual rows per core
_NBLK = _ROWS // 128                    # 8 blocks of 128 partitions
_CW = 20                                # coord cols per block (10 x | 10 y)
_CF = _NBLK * _CW                       # 160 coord cols total
_PF = _NBLK * 100                       # 800 product cols total
_MAGIC = 8388608.0                      # 2^23 round-to-nearest trick
_BIG = 8192.0                           # 2^13 mask scale

LAST_RESULTS = None  # BassKernelResults of the most recent run (for test.py)

_compiled = {}


def _build_nc():
    import concourse.bass as bass
    import concourse.bacc as bacc
    import concourse.tile as tile
    from concourse import mybir

    f32 = mybir.dt.float32
    bf16 = mybir.dt.bfloat16
    Op = mybir.AluOpType
    Act = mybir.ActivationFunctionType

    nc = bacc.Bacc("TRN2", target_bir_lowering=False, debug=False,
                   num_devices=_N_CORES)
    # wts layout: cols [0, 1024) = T lhsT per block (rows 0-7: hi/lo
    # pairs); [1024, 1064) = rhs (first 20 for U/V/Un rows 0-3, last 20 for
    # T rows 0-7); [1064, 4136) = per block U|V|Un lhsT (rows 0-3).
    wts_d = nc.dram_tensor("wts", [8, _NBLK * 512 + 40], bf16,
                           kind="ExternalInput").ap()
    s2t_d = nc.dram_tensor("s2t", [128, _CF], bf16, kind="ExternalInput").ap()
    out_d = nc.dram_tensor("out", [2, 1], f32, kind="ExternalOutput").ap()

    with tile.TileContext(nc) as tc:
        with (
            tc.tile_pool(name="const", bufs=1) as const,
            tc.tile_pool(name="sb", bufs=1) as sb,
            tc.tile_pool(name="ps", bufs=1, space="PSUM") as ps,
        ):
            wts = const.tile([8, _NBLK * 512 + 40], bf16)
            s2t = const.tile([128, _CF], bf16)
            # T lhsT + rhs land first: the T->kc0 chain is critical
            TW = _NBLK * 128
            nc.sync.dma_start(wts[0:8, 0:TW + 40], wts_d[0:8, 0:TW + 40])
            nc.sync.dma_start(wts[0:4, TW + 40:], wts_d[0:4, TW + 40:])
            nc.sync.dma_start(s2t[:], s2t_d[:])
            cb = const.tile([128, 1], f32)
            nc.gpsimd.memset(cb[:], -_MAGIC)
            ones = const.tile([128, 1], f32)
            nc.gpsimd.memset(ones[:], 1.0)
            part = const.tile([128, 2], f32)

            U = ps.tile([128, _CF], f32, tag="U")
            V = ps.tile([128, _CF], f32, tag="V")
            Un = ps.tile([128, _CF], f32, tag="Un")
            T = ps.tile([128, _CF], f32, tag="T")
            TW = _NBLK * 128
            for b in range(_NBLK):
                cs = slice(b * _CW, (b + 1) * _CW)
                nc.tensor.matmul(T[:, cs], wts[0:8, b * 128:(b + 1) * 128],
                                 wts[0:8, TW + 20:TW + 40])
            rr = slice(TW, TW + 20)
            for b in range(_NBLK):
                cs = slice(b * _CW, (b + 1) * _CW)
                l0 = TW + 40 + b * 384
                nc.tensor.matmul(U[:, cs], wts[0:4, l0:l0 + 128], wts[0:4, rr])
                nc.tensor.matmul(V[:, cs], wts[0:4, l0 + 128:l0 + 256],
                                 wts[0:4, rr])
                nc.tensor.matmul(Un[:, cs], wts[0:4, l0 + 256:l0 + 384],
                                 wts[0:4, rr])

            # --- coordinate-level ops, [128, 160] ---
            # k = clamp(round(T), 0, 24); d = k - T; val^2 = s^2 * d^2
            kc0 = sb.tile([128, _CF], f32, tag="kc0")
            nc.vector.tensor_scalar(kc0[:], T[:], _MAGIC, _MAGIC + 24.0,
                                    Op.add, Op.min)
            kc1 = sb.tile([128, _CF], bf16, tag="kc1")
            nc.scalar.activation(kc1[:], kc0[:], Act.Relu, bias=cb[:, 0:1])
            d = sb.tile([128, _CF], bf16, tag="d")
            nc.vector.tensor_tensor(d[:], kc1[:], T[:], Op.subtract)
            dsq = sb.tile([128, _CF], bf16, tag="dsq")
            nc.scalar.activation(dsq[:], d[:], Act.Square)
            vq = sb.tile([128, _CF], bf16, tag="vq")
            nc.vector.tensor_tensor(vq[:], dsq[:], s2t[:], Op.mult)

            usq = sb.tile([128, _CF], bf16, tag="usq")
            nc.scalar.activation(usq[:], U[:], Act.Square)
            vsq = sb.tile([128, _CF], bf16, tag="vsq")
            nc.scalar.activation(vsq[:], V[:], Act.Square)
            exy = sb.tile([128, _CF], bf16, tag="exy")
            nc.vector.tensor_tensor(exy[:], usq[:], vsq[:], Op.min)

            # mask path: m1n = max(-BIG*u, -BIG*v) = -BIG*min(u,v)
            vsn = sb.tile([128, _CF], bf16, tag="vsn")
            nc.scalar.activation(vsn[:], V[:], Act.Copy, scale=-_BIG)
            m1n = sb.tile([128, _CF], bf16, tag="m1n")
            nc.vector.tensor_tensor(m1n[:], Un[:], vsn[:], Op.max)

            # --- fragment-product ops, [128, 800] via broadcast views ---
            def cview(t, c, inner_j):
                # [128, 8, 10, 10] view of coord half c (0=x grid i, 1=y grid j)
                a = t[:].rearrange("p (b c t) -> p b c t", b=_NBLK, c=2)
                a = a[:, :, c, :]              # [128, 8, 10]
                if inner_j:                    # values indexed by j (inner)
                    a = a.unsqueeze(2)         # [128, 8, 1, 10]
                else:                          # values indexed by i (outer)
                    a = a.unsqueeze(3)         # [128, 8, 10, 1]
                return a.broadcast_to((128, _NBLK, 10, 10))

            def pview(t):
                return t[:].rearrange("p (b i j) -> p b i j", i=10, j=10)

            # mmx = relu(-BIG * min over the 4 coord margins); the relu
            # rides the Act-materialized packed x-repeat (relu(max(a,b)) ==
            # max(b, relu(a))), so the max runs in 2x DVE mode.  Since
            # dmin >= 0, contribution = relu(min(e1,e2,raw)) = min(dmin,mmx).
            e1 = sb.tile([128, _PF], bf16, tag="e1")
            nc.vector.tensor_tensor(pview(e1), cview(vq, 1, True),
                                    cview(exy, 0, False), Op.add)
            e2 = sb.tile([128, _PF], bf16, tag="e2")
            nc.vector.tensor_tensor(pview(e2), cview(vq, 0, False),
                                    cview(exy, 1, True), Op.add)
            m1xr = sb.tile([128, _PF], bf16, tag="m1xr")
            nc.scalar.activation(pview(m1xr), cview(m1n, 0, False), Act.Relu)
            mmx = sb.tile([128, _PF], bf16, tag="mmx")
            nc.vector.tensor_tensor(pview(mmx), cview(m1n, 1, True),
                                    pview(m1xr), Op.max)
            dmin = sb.tile([128, _PF], bf16, tag="dmin")
            nc.vector.tensor_tensor(dmin[:], e1[:], e2[:], Op.min)
            tm = sb.tile([128, _PF], bf16, tag="tm")
            nc.vector.tensor_tensor(tm[:], dmin[:], mmx[:], Op.min)
            # final sums: halves on different engines so they run in parallel
            scr = sb.tile([128, _PF], bf16, tag="scr")
            nc.scalar.activation(scr[:, 0:_PF // 2], tm[:, 0:_PF // 2],
                                 Act.Identity, accum_out=part[:, 0:1])
            nc.vector.tensor_scalar(scr[:, _PF // 2:], tm[:, _PF // 2:], 0.0,
                                    None, Op.add, Op.add,
                                    accum_out=part[:, 1:2])

            # collapse 128 per-partition partials -> [1,1] so the output DMA
            # is a single descriptor
            sm = ps.tile([2, 1], f32, tag="sm")
            nc.tensor.matmul(sm[0:2, 0:1], part[:, 0:2], ones[:, 0:1])
            smc = sb.tile([2, 1], f32, tag="smc")
            nc.scalar.activation(smc[0:2, 0:1], sm[0:2, 0:1], Act.Copy)
            nc.sync.dma_start(out_d[:], smc[0:2, 0:1])
    nc.compile()
    return nc


def _bf16_hilo(v):
    bf = ml_dtypes.bfloat16
    hi = v.astype(bf)
    lo = (v - hi.astype(np.float32)).astype(bf)
    return hi, lo


def _core_inputs(boxes_c, targets_c):
    """Build the per-core DRAM input map (512 boxes -> 1024 virtual rows)."""
    A = np.concatenate([boxes_c, targets_c]).astype(np.float32)   # frag source
    T = np.concatenate([targets_c, boxes_c]).astype(np.float32)   # grid box
    w = A[:, 2] - A[:, 0]
    h = A[:, 3] - A[:, 1]
    tw = T[:, 2] - T[:, 0]
    th = T[:, 3] - T[:, 1]
    dx = A[:, 0] - T[:, 0]
    dy = A[:, 1] - T[:, 1]
    dvx = T[:, 2] - A[:, 0]
    dvy = T[:, 3] - A[:, 1]
    with np.errstate(divide="ignore"):
        rix = np.where(tw != 0, np.float32(24.0) / tw, np.float32(0.0))
        riy = np.where(th != 0, np.float32(24.0) / th, np.float32(0.0))
    rix = rix.astype(np.float32)
    riy = riy.astype(np.float32)
    nbig = np.float32(-_BIG)

    bf = ml_dtypes.bfloat16
    wts = np.zeros((8, _NBLK * 512 + 40), dtype=np.float32)
    TW = _NBLK * 128
    for b in range(_NBLK):
        rs = slice(b * 128, (b + 1) * 128)
        l0 = TW + 40 + b * 384
        wts[0, l0:l0 + 128] = w[rs]
        wts[1, l0:l0 + 128] = dx[rs]
        wts[2, l0:l0 + 128] = h[rs]
        wts[3, l0:l0 + 128] = dy[rs]
        wts[0, l0 + 128:l0 + 256] = -w[rs]
        wts[1, l0 + 128:l0 + 256] = dvx[rs]
        wts[2, l0 + 128:l0 + 256] = -h[rs]
        wts[3, l0 + 128:l0 + 256] = dvy[rs]
        wts[0, l0 + 256:l0 + 384] = nbig * w[rs]
        wts[1, l0 + 256:l0 + 384] = nbig * dx[rs]
        wts[2, l0 + 256:l0 + 384] = nbig * h[rs]
        wts[3, l0 + 256:l0 + 384] = nbig * dy[rs]
    wtsb = wts.astype(bf)
    for b in range(_NBLK):
        rs = slice(b * 128, (b + 1) * 128)
        t0 = b * 128
        for row, v in ((0, w * rix), (2, dx * rix),
                       (4, h * riy), (6, dy * riy)):
            hi, lo = _bf16_hilo(v[rs].astype(np.float32))
            wtsb[row, t0:t0 + 128] = hi
            wtsb[row + 1, t0:t0 + 128] = lo

    rhs = np.zeros((8, 40), dtype=np.float32)
    rhs[0, 0:10] = _LIN10
    rhs[1, 0:10] = 1.0
    rhs[2, 10:20] = _LIN10
    rhs[3, 10:20] = 1.0
    rhs[0, 20:30] = _LIN10
    rhs[1, 20:30] = _LIN10
    rhs[2, 20:30] = 1.0
    rhs[3, 20:30] = 1.0
    rhs[4, 30:40] = _LIN10
    rhs[5, 30:40] = _LIN10
    rhs[6, 30:40] = 1.0
    rhs[7, 30:40] = 1.0
    wtsb[:, TW:TW + 40] = rhs.astype(bf)

    sx = tw / np.float32(24.0)
    sy = th / np.float32(24.0)
    s2 = np.zeros((128, _NBLK, 2, 10), dtype=np.float32)
    for b in range(_NBLK):
        rs = slice(b * 128, (b + 1) * 128)
        s2[:, b, 0, :] = (sx[rs] * sx[rs])[:, None]
        s2[:, b, 1, :] = (sy[rs] * sy[rs])[:, None]

    return {
        "wts": wtsb,
        "s2t": s2.reshape(128, _CF).astype(bf),
    }


def kernel(boxes: np.ndarray, targets: np.ndarray) -> np.ndarray:
    from concourse.bass_utils import run_bass_kernel_spmd

    global LAST_RESULTS
    boxes = np.ascontiguousarray(boxes, dtype=np.float32)
    targets = np.ascontiguousarray(targets, dtype=np.float32)
    assert boxes.shape == (_B, 4) and targets.shape == (_B, 4)

    if "nc" not in _compiled:
        _compiled["nc"] = _build_nc()
    nc = _compiled["nc"]

    in_maps = []
    for c in range(_N_CORES):
        rows = slice(c * _BOX_PER_CORE, (c + 1) * _BOX_PER_CORE)
        in_maps.append(_core_inputs(boxes[rows], targets[rows]))

    trace = bool(int(os.environ.get("BOXLOSS_TRACE", "0")))
    res = run_bass_kernel_spmd(nc, in_maps, list(range(_N_CORES)),
                               trace=trace)
    LAST_RESULTS = res

    total = np.float64(0.0)
    for r in res.results:
        total += r["out"].astype(np.float64).sum()
    loss = total / (2.0 * _B * _FP)
    return np.array(loss, dtype=np.float32)


# revision 15
# speedup vs baseline: 1.3902x; 1.0214x over previous
"""BoxRenderLoss Trainium2 kernel (v4: rows-on-partitions, 10-pt coord grids).

loss = mean over (box, fragment) pairs of masked min-squared-distance between
each box's 10x10 fragment grid and the other box's 100-point sampled boundary,
both directions, / (2*B*FP).

Math: the min over the 100 boundary points decomposes into the 4 box edges;
each edge's 25-point uniform grid min has the closed form
k = clamp(round(u/s), 0, 24), val = u - s*k = -s*(k - T), T = u/s.  Key
structural win: the fragment grid is a 10x10 product grid, so every
per-coordinate quantity (u, v, u^2, v^2, val^2, min(u,v)) takes only 10
distinct values per axis per row; only the final combine runs over the full
100 fragments, via stride-0 broadcast access patterns.

The outside-mask multiply is folded into the min-chain:
  contribution = relu(min(e1, e2, -BIG*min(ux,vx,uy,vy)))
with BIG = 2^13: inside fragments give a nonpositive third term -> relu -> 0;
outside ones a huge positive one -> min picks dmin.  The final relu+row-sum is
a 4x-mode DVE tensor_scalar with accum_out; a K=128 PE matmul against a ones
vector then collapses the 128 partials so the output DMA is one descriptor
(a [128,1] output costs ~9us of per-descriptor semaphore propagation).

All matmul weights are bf16 in ONE DMA; T = u*(24/tw) needs ~f32 accuracy
(wrx, dxrx cancel), so its lhsT carries hi/lo bf16 pairs (K=8 split matmul),
which beats f32r's 4-pass LdWeights+Matmult (213+213ns per block) ~5x.

Device layout: partitions = 128 virtual rows (4096 boxes x 2 directions,
data-parallel over 8 cores -> 1024 rows/core = 8 blocks of 128), free dim =
8 blocks x (10 x-grid | 10 y-grid) = 160 for coordinate ops, 8 x 10 x 10 =
800 for fragment-product ops.
"""

import os
import numpy as np
import ml_dtypes

# Exact float32 bit patterns of jnp.linspace(0.0, 1.0, 10) (fragment grid).
_LIN10 = np.array(
    [0, 1038323257, 1046711865, 1051372203, 1055100473,
     1057896676, 1059760811, 1061624946, 1063489081, 1065353216],
    dtype=np.uint32,
).view(np.float32)

_B = 4096
_FP = 100
_N_CORES = 8
_BOX_PER_CORE = _B // _N_CORES          # 512
_ROWS = 2 * _BOX_PER_CORE               # 1024 virtual rows per core
_NBLK = _ROWS // 128                    # 8 blocks of 128 partitions
_CW = 20                                # coord cols per block (10 x | 10 y)
_CF = _NBLK * _CW                       # 160 coord cols total
_PF = _NBLK * 100                       # 800 product cols total
_MAGIC = 8388608.0                      # 2^23 round-to-nearest trick
_BIG = 8192.0                           # 2^13 mask scale

LAST_RESULTS = None  # BassKernelResults of the most recent run (for test.py)

_compiled = {}


def _build_nc():
    import concourse.bass as bass
    import concourse.bacc as bacc
    import concourse.tile as tile
    from concourse import mybir

    f32 = mybir.dt.float32
    bf16 = mybir.dt.bfloat16
    Op = mybir.AluOpType
    Act = mybir.ActivationFunctionType

    nc = bacc.Bacc("TRN2", target_bir_lowering=False, debug=False,
                   num_devices=_N_CORES)
    # wts layout: cols [0, 1024) = T lhsT per block (rows 0-7: hi/lo
    # pairs); [1024, 1064) = rhs (first 20 for U/V/Un rows 0-3, last 20 for
    # T rows 0-7); [1064, 4136) = per block U|V|Un lhsT (rows 0-3).
    wts_d = nc.dram_tensor("wts", [8, _NBLK * 512 + 40], bf16,
                           kind="ExternalInput").ap()
    s2t_d = nc.dram_tensor("s2t", [128, _CF], bf16, kind="ExternalInput").ap()
    out_d = nc.dram_tensor("out", [2, 1], f32, kind="ExternalOutput").ap()

    with tile.TileContext(nc) as tc:
        with (
            tc.tile_pool(name="const", bufs=1) as const,
            tc.tile_pool(name="sb", bufs=1) as sb,
            tc.tile_pool(name="ps", bufs=1, space="PSUM") as ps,
        ):
            wts = const.tile([8, _NBLK * 512 + 40], bf16)
            s2t = const.tile([128, _CF], bf16)
            # T lhsT + rhs land first: the T->kc0 chain is critical
            TW = _NBLK * 128
            # one DMA per engine DGE so the three setups run in parallel
            nc.sync.dma_start(wts[0:8, 0:TW + 40], wts_d[0:8, 0:TW + 40])
            nc.scalar.dma_start(wts[0:4, TW + 40:], wts_d[0:4, TW + 40:])
            nc.gpsimd.dma_start(s2t[:], s2t_d[:])
            cb = const.tile([128, 1], f32)
            nc.gpsimd.memset(cb[:], -_MAGIC)
            ones = const.tile([128, 1], f32)
            nc.gpsimd.memset(ones[:], 1.0)
            part = const.tile([128, 2], f32)
            # tiny warm-up activation: hoists the ACT table load off the
            # critical path (it otherwise waits for the first real Act op's
            # input, which sits behind the weight DMA + matmuls)
            warm = const.tile([1, 1], f32)
            nc.scalar.activation(warm[0:1, 0:1], ones[0:1, 0:1], Act.Square)

            U = ps.tile([128, _CF], f32, tag="U")
            V = ps.tile([128, _CF], f32, tag="V")
            Un = ps.tile([128, _CF], f32, tag="Un")
            T = ps.tile([128, _CF], f32, tag="T")
            TW = _NBLK * 128
            for b in range(_NBLK):
                cs = slice(b * _CW, (b + 1) * _CW)
                nc.tensor.matmul(T[:, cs], wts[0:8, b * 128:(b + 1) * 128],
                                 wts[0:8, TW + 20:TW + 40])
            rr = slice(TW, TW + 20)
            for b in range(_NBLK):
                cs = slice(b * _CW, (b + 1) * _CW)
                l0 = TW + 40 + b * 384
                nc.tensor.matmul(U[:, cs], wts[0:4, l0:l0 + 128], wts[0:4, rr])
                nc.tensor.matmul(V[:, cs], wts[0:4, l0 + 128:l0 + 256],
                                 wts[0:4, rr])
                nc.tensor.matmul(Un[:, cs], wts[0:4, l0 + 256:l0 + 384],
                                 wts[0:4, rr])

            # --- coordinate-level ops, [128, 160] ---
            # k = clamp(round(T), 0, 24); d = k - T; val^2 = s^2 * d^2
            kc0 = sb.tile([128, _CF], f32, tag="kc0")
            nc.vector.tensor_scalar(kc0[:], T[:], _MAGIC, _MAGIC + 24.0,
                                    Op.add, Op.min)
            kc1 = sb.tile([128, _CF], bf16, tag="kc1")
            nc.scalar.activation(kc1[:], kc0[:], Act.Relu, bias=cb[:, 0:1])
            d = sb.tile([128, _CF], bf16, tag="d")
            nc.vector.tensor_tensor(d[:], kc1[:], T[:], Op.subtract)
            dsq = sb.tile([128, _CF], bf16, tag="dsq")
            nc.scalar.activation(dsq[:], d[:], Act.Square)
            vq = sb.tile([128, _CF], bf16, tag="vq")
            nc.vector.tensor_tensor(vq[:], dsq[:], s2t[:], Op.mult)

            usq = sb.tile([128, _CF], bf16, tag="usq")
            nc.scalar.activation(usq[:], U[:], Act.Square)
            vsq = sb.tile([128, _CF], bf16, tag="vsq")
            nc.scalar.activation(vsq[:], V[:], Act.Square)
            exy = sb.tile([128, _CF], bf16, tag="exy")
            nc.vector.tensor_tensor(exy[:], usq[:], vsq[:], Op.min)

            # mask path: m1n = max(-BIG*u, -BIG*v) = -BIG*min(u,v)
            vsn = sb.tile([128, _CF], bf16, tag="vsn")
            nc.scalar.activation(vsn[:], V[:], Act.Copy, scale=-_BIG)
            m1n = sb.tile([128, _CF], bf16, tag="m1n")
            nc.vector.tensor_tensor(m1n[:], Un[:], vsn[:], Op.max)

            # --- fragment-product ops, [128, 800] via broadcast views ---
            def cview(t, c, inner_j):
                # [128, 8, 10, 10] view of coord half c (0=x grid i, 1=y grid j)
                a = t[:].rearrange("p (b c t) -> p b c t", b=_NBLK, c=2)
                a = a[:, :, c, :]              # [128, 8, 10]
                if inner_j:                    # values indexed by j (inner)
                    a = a.unsqueeze(2)         # [128, 8, 1, 10]
                else:                          # values indexed by i (outer)
                    a = a.unsqueeze(3)         # [128, 8, 10, 1]
                return a.broadcast_to((128, _NBLK, 10, 10))

            def pview(t):
                return t[:].rearrange("p (b i j) -> p b i j", i=10, j=10)

            # mmx = relu(-BIG * min over the 4 coord margins); the relu
            # rides the Act-materialized packed x-repeat (relu(max(a,b)) ==
            # max(b, relu(a))), so the max runs in 2x DVE mode.  Since
            # dmin >= 0, contribution = relu(min(e1,e2,raw)) = min(dmin,mmx).
            e1 = sb.tile([128, _PF], bf16, tag="e1")
            nc.vector.tensor_tensor(pview(e1), cview(vq, 1, True),
                                    cview(exy, 0, False), Op.add)
            e2 = sb.tile([128, _PF], bf16, tag="e2")
            nc.vector.tensor_tensor(pview(e2), cview(vq, 0, False),
                                    cview(exy, 1, True), Op.add)
            m1xr = sb.tile([128, _PF], bf16, tag="m1xr")
            nc.scalar.activation(pview(m1xr), cview(m1n, 0, False), Act.Relu)
            mmx = sb.tile([128, _PF], bf16, tag="mmx")
            nc.vector.tensor_tensor(pview(mmx), cview(m1n, 1, True),
                                    pview(m1xr), Op.max)
            dmin = sb.tile([128, _PF], bf16, tag="dmin")
            nc.vector.tensor_tensor(dmin[:], e1[:], e2[:], Op.min)
            tm = sb.tile([128, _PF], bf16, tag="tm")
            nc.vector.tensor_tensor(tm[:], dmin[:], mmx[:], Op.min)
            # final sums: halves on different engines so they run in parallel
            scr = sb.tile([128, _PF], bf16, tag="scr")
            nc.scalar.activation(scr[:, 0:_PF // 2], tm[:, 0:_PF // 2],
                                 Act.Identity, accum_out=part[:, 0:1])
            nc.vector.tensor_scalar(scr[:, _PF // 2:], tm[:, _PF // 2:], 0.0,
                                    None, Op.add, Op.add,
                                    accum_out=part[:, 1:2])

            # collapse 128 per-partition partials -> [1,1] so the output DMA
            # is a single descriptor
            sm = ps.tile([2, 1], f32, tag="sm")
            nc.tensor.matmul(sm[0:2, 0:1], part[:, 0:2], ones[:, 0:1])
            smc = sb.tile([2, 1], f32, tag="smc")
            nc.scalar.activation(smc[0:2, 0:1], sm[0:2, 0:1], Act.Copy)
            nc.sync.dma_start(out_d[:], smc[0:2, 0:1])
    nc.compile()
    return nc


def _bf16_hilo(v):
    bf = ml_dtypes.bfloat16
    hi = v.astype(bf)
    lo = (v - hi.astype(np.float32)).astype(bf)
    return hi, lo


def _core_inputs(boxes_c, targets_c):
    """Build the per-core DRAM input map (512 boxes -> 1024 virtual rows)."""
    A = np.concatenate([boxes_c, targets_c]).astype(np.float32)   # frag source
    T = np.concatenate([targets_c, boxes_c]).astype(np.float32)   # grid box
    w = A[:, 2] - A[:, 0]
    h = A[:, 3] - A[:, 1]
    tw = T[:, 2] - T[:, 0]
    th = T[:, 3] - T[:, 1]
    dx = A[:, 0] - T[:, 0]
    dy = A[:, 1] - T[:, 1]
    dvx = T[:, 2] - A[:, 0]
    dvy = T[:, 3] - A[:, 1]
    with np.errstate(divide="ignore"):
        rix = np.where(tw != 0, np.float32(24.0) / tw, np.float32(0.0))
        riy = np.where(th != 0, np.float32(24.0) / th, np.float32(0.0))
    rix = rix.astype(np.float32)
    riy = riy.astype(np.float32)
    nbig = np.float32(-_BIG)

    bf = ml_dtypes.bfloat16
    wts = np.zeros((8, _NBLK * 512 + 40), dtype=np.float32)
    TW = _NBLK * 128
    for b in range(_NBLK):
        rs = slice(b * 128, (b + 1) * 128)
        l0 = TW + 40 + b * 384
        wts[0, l0:l0 + 128] = w[rs]
        wts[1, l0:l0 + 128] = dx[rs]
        wts[2, l0:l0 + 128] = h[rs]
        wts[3, l0:l0 + 128] = dy[rs]
        wts[0, l0 + 128:l0 + 256] = -w[rs]
        wts[1, l0 + 128:l0 + 256] = dvx[rs]
        wts[2, l0 + 128:l0 + 256] = -h[rs]
        wts[3, l0 + 128:l0 + 256] = dvy[rs]
        wts[0, l0 + 256:l0 + 384] = nbig * w[rs]
        wts[1, l0 + 256:l0 + 384] = nbig * dx[rs]
        wts[2, l0 + 256:l0 + 384] = nbig * h[rs]
        wts[3, l0 + 256:l0 + 384] = nbig * dy[rs]
    wtsb = wts.astype(bf)
    for b in range(_NBLK):
        rs = slice(b * 128, (b + 1) * 128)
        t0 = b * 128
        for row, v in ((0, w * rix), (2, dx * rix),
                       (4, h * riy), (6, dy * riy)):
            hi, lo = _bf16_hilo(v[rs].astype(np.float32))
            wtsb[row, t0:t0 + 128] = hi
            wtsb[row + 1, t0:t0 + 128] = lo

    rhs = np.zeros((8, 40), dtype=np.float32)
    rhs[0, 0:10] = _LIN10
    rhs[1, 0:10] = 1.0
    rhs[2, 10:20] = _LIN10
    rhs[3, 10:20] = 1.0
    rhs[0, 20:30] = _LIN10
    rhs[1, 20:30] = _LIN10
    rhs[2, 20:30] = 1.0
    rhs[3, 20:30] = 1.0
    rhs[4, 30:40] = _LIN10
    rhs[5, 30:40] = _LIN10
    rhs[6, 30:40] = 1.0
    rhs[7, 30:40] = 1.0
    wtsb[:, TW:TW + 40] = rhs.astype(bf)

    sx = tw / np.float32(24.0)
    sy = th / np.float32(24.0)
    s2 = np.zeros((128, _NBLK, 2, 10), dtype=np.float32)
    for b in range(_NBLK):
        rs = slice(b * 128, (b + 1) * 128)
        s2[:, b, 0, :] = (sx[rs] * sx[rs])[:, None]
        s2[:, b, 1, :] = (sy[rs] * sy[rs])[:, None]

    return {
        "wts": wtsb,
        "s2t": s2.reshape(128, _CF).astype(bf),
    }


def kernel(boxes: np.ndarray, targets: np.ndarray) -> np.ndarray:
    from concourse.bass_utils import run_bass_kernel_spmd

    global LAST_RESULTS
    boxes = np.ascontiguousarray(boxes, dtype=np.float32)
    targets = np.ascontiguousarray(targets, dtype=np.float32)
    assert boxes.shape == (_B, 4) and targets.shape == (_B, 4)

    if "nc" not in _compiled:
        _compiled["nc"] = _build_nc()
    nc = _compiled["nc"]

    in_maps = []
    for c in range(_N_CORES):
        rows = slice(c * _BOX_PER_CORE, (c + 1) * _BOX_PER_CORE)
        in_maps.append(_core_inputs(boxes[rows], targets[rows]))

    trace = bool(int(os.environ.get("BOXLOSS_TRACE", "0")))
    res = run_bass_kernel_spmd(nc, in_maps, list(range(_N_CORES)),
                               trace=trace)
    LAST_RESULTS = res

    total = np.float64(0.0)
    for r in res.results:
        total += r["out"].astype(np.float64).sum()
    loss = total / (2.0 * _B * _FP)
    return np.array(loss, dtype=np.float32)
